# revision 2
# baseline (speedup 1.0000x reference)
"""Multi-head self-attention on 8 Trainium2 NeuronCores.

Tensor-parallel over heads: core c owns heads 2c, 2c+1 (128 of the 1024
hidden columns).  Each core:
  1. Qt/Kt = (x @ wq/wk + b)^T  in [d, token] layout (d on partitions,
     2 heads stacked: partitions 0:64 head0, 64:128 head1)
  2. V_aug = x @ [wv | 0] + [bv | 1]  in [token, 65-per-head] layout
     (ones column -> softmax denominator comes out of the P@V matmul)
  3. per (batch, head): S^T tiles = K^T.T @ Q^T  (contraction d=64),
     P^T = exp(S^T / 8) (no max subtraction needed: |S/8| < ~3),
     out^T[65, s] = V_aug.T @ P^T accumulated over t tiles,
     attnT = out^T[0:64] * broadcast(1 / out^T[64])
  4. partial = attnT.T @ wo[128 rows of this core]  -> HBM (f32)
Host sums the 8 partials and adds bo.

Shapes hardcoded for x:[2,2048,1024], 16 heads, d_k=64.
"""

import numpy as np
import ml_dtypes

import concourse.bass as bass
import concourse.tile as tile
from concourse import bacc, mybir
from concourse.bass import ts
from concourse.bass_utils import run_bass_kernel_spmd

BF16 = mybir.dt.bfloat16
F32 = mybir.dt.float32
NPBF16 = ml_dtypes.bfloat16

B = 2
S = 2048
D = 1024
NT = B * S  # 4096 tokens
DK = 64
NCORES = 8
HPC = 2  # heads per core
SC = 1024  # attention s-chunk (exp op free size)

_CACHE = {}


def _build_nc():
    nc = bacc.Bacc("TRN2", target_bir_lowering=False, debug=False,
                   num_devices=NCORES)

    xT = nc.dram_tensor("xT", [D, NT], BF16, kind="ExternalInput").ap()
    wq = nc.dram_tensor("wq", [D, 128], BF16, kind="ExternalInput").ap()
    wk = nc.dram_tensor("wk", [D, 128], BF16, kind="ExternalInput").ap()
    wv = nc.dram_tensor("wv", [D, 130], BF16, kind="ExternalInput").ap()
    bq = nc.dram_tensor("bq", [128, 1], F32, kind="ExternalInput").ap()
    bk = nc.dram_tensor("bk", [128, 1], F32, kind="ExternalInput").ap()
    bv = nc.dram_tensor("bv", [128, 130], F32, kind="ExternalInput").ap()
    wo = nc.dram_tensor("wo", [128, D], BF16, kind="ExternalInput").ap()
    out = nc.dram_tensor("out", [NT, D], F32, kind="ExternalOutput").ap()

    with tile.TileContext(nc) as tc:
        _emit(nc, tc, xT, wq, wk, wv, bq, bk, bv, wo, out)
    nc.compile()
    return nc


def _emit(nc, tc, xT, wq, wk, wv, bq, bk, bv, wo, out):
    import contextlib
    ctx = contextlib.ExitStack()
    with ctx:
        consts = ctx.enter_context(tc.tile_pool(name="consts", bufs=1))
        ptp = ctx.enter_context(tc.tile_pool(name="ptp", bufs=34))
        psp = ctx.enter_context(tc.tile_pool(name="psp", bufs=4, space="PSUM"))
        stg = ctx.enter_context(tc.tile_pool(name="stg", bufs=4))
        nrm = ctx.enter_context(tc.tile_pool(name="nrm", bufs=3))

        # ---- persistent SBUF tensors ----
        xT_sb = consts.tile([128, 8, NT], BF16)      # 8 k-tiles of x^T
        wq_sb = consts.tile([128, 8, 128], BF16)
        wk_sb = consts.tile([128, 8, 128], BF16)
        wv_sb = consts.tile([128, 8, 130], BF16)
        bq_sb = consts.tile([128, 1], F32)
        bk_sb = consts.tile([128, 1], F32)
        bv_sb = consts.tile([128, 130], F32)
        wo_sb = consts.tile([128, D], BF16)
        QT = consts.tile([128, NT], BF16)
        KT = consts.tile([128, NT], BF16)
        V_sb = consts.tile([128, 32, 130], BF16)     # [t-in-tile, t_tile, col]
        attnT = consts.tile([128, NT], BF16)

        xT_d = xT.rearrange("(k p) n -> k p n", p=128)
        wq_d = wq.rearrange("(k p) c -> k p c", p=128)
        wk_d = wk.rearrange("(k p) c -> k p c", p=128)
        wv_d = wv.rearrange("(k p) c -> k p c", p=128)
        for k in range(8):
            nc.sync.dma_start(out=xT_sb[:, k, :], in_=xT_d[k])
            nc.sync.dma_start(out=wq_sb[:, k, :], in_=wq_d[k])
            nc.sync.dma_start(out=wk_sb[:, k, :], in_=wk_d[k])
            nc.sync.dma_start(out=wv_sb[:, k, :], in_=wv_d[k])
        nc.sync.dma_start(out=bq_sb, in_=bq)
        nc.sync.dma_start(out=bk_sb, in_=bk)
        nc.sync.dma_start(out=bv_sb, in_=bv)
        nc.sync.dma_start(out=wo_sb, in_=wo)

        # ---- Q^T / K^T projections: [c,128] += w[k].T @ xT[k] ----
        for w_sb, b_sb, o_sb in ((wq_sb, bq_sb, QT), (wk_sb, bk_sb, KT)):
            for n in range(8):  # 512-token chunks
                ps = psp.tile([128, 512], F32, tag="ps")
                for k in range(8):
                    nc.tensor.matmul(ps, lhsT=w_sb[:, k, :],
                                     rhs=xT_sb[:, k, ts(n, 512)],
                                     start=(k == 0), stop=(k == 7))
                nc.scalar.activation(out=o_sb[:, ts(n, 512)], in_=ps,
                                     func=mybir.ActivationFunctionType.Identity,
                                     bias=b_sb, scale=1.0)

        # ---- V_aug projection: [t,130] += xT[k,t-tile].T @ wv[k] ----
        for tt in range(32):
            psv = psp.tile([128, 130], F32, tag="ps")
            for k in range(8):
                nc.tensor.matmul(psv, lhsT=xT_sb[:, k, ts(tt, 128)],
                                 rhs=wv_sb[:, k, :],
                                 start=(k == 0), stop=(k == 7))
            nc.vector.tensor_add(V_sb[:, tt, :], psv, bv_sb)

        # ---- attention + output projection ----
        for b in range(B):
            for sc in range(S // SC):
                s0 = b * S + sc * SC
                pts = []
                for tt in range(16):
                    row = []
                    for h in range(HPC):
                        ps = psp.tile([128, SC], F32, tag="ps")
                        hsl = slice(h * DK, (h + 1) * DK)
                        for n2 in range(SC // 512):
                            nc.tensor.matmul(
                                ps[:, ts(n2, 512)],
                                lhsT=KT[hsl, b * S + tt * 128:b * S + (tt + 1) * 128],
                                rhs=QT[hsl, s0 + n2 * 512:s0 + (n2 + 1) * 512],
                                start=True, stop=True)
                        pt = ptp.tile([128, SC], BF16, tag="pt")
                        nc.scalar.activation(
                            out=pt, in_=ps,
                            func=mybir.ActivationFunctionType.Exp,
                            scale=0.125)
                        row.append(pt)
                    pts.append(row)
                for h in range(HPC):
                    pso = psp.tile([128, SC], F32, tag="ps")
                    for tt in range(16):
                        for n2 in range(SC // 512):
                            nc.tensor.matmul(
                                pso[0:65, ts(n2, 512)],
                                lhsT=V_sb[:, b * 16 + tt, h * 65:(h + 1) * 65],
                                rhs=pts[tt][h][:, ts(n2, 512)],
                                start=(tt == 0), stop=(tt == 15))
                    rec = nrm.tile([1, SC], F32, tag="rec")
                    nc.vector.reciprocal(out=rec, in_=pso[64:65, :])
                    recb = nrm.tile([64, SC], F32, tag="recb")
                    nc.gpsimd.partition_broadcast(recb, rec)
                    nc.vector.tensor_mul(
                        attnT[h * DK:(h + 1) * DK, s0:s0 + SC],
                        pso[0:64, :], recb)
            # output projection for this batch's tokens
            for tt in range(b * 16, (b + 1) * 16):
                for eh in range(2):
                    pw = psp.tile([128, 512], F32, tag="ps")
                    nc.tensor.matmul(pw, lhsT=attnT[:, ts(tt, 128)],
                                     rhs=wo_sb[:, ts(eh, 512)],
                                     start=True, stop=True)
                    ob = stg.tile([128, 512], F32, tag="ob")
                    nc.vector.tensor_copy(ob, pw)
                    nc.sync.dma_start(
                        out=out[tt * 128:(tt + 1) * 128, eh * 512:(eh + 1) * 512],
                        in_=ob)


def _prep_in_maps(x, wq, bq, wk, bk, wv, bv, wo):
    x2 = np.asarray(x, np.float32).reshape(NT, D)
    xT = np.ascontiguousarray(x2.T).astype(NPBF16)
    wq = np.asarray(wq, np.float32)
    wk = np.asarray(wk, np.float32)
    wv = np.asarray(wv, np.float32)
    wo = np.asarray(wo, np.float32)
    bq = np.asarray(bq, np.float32)
    bk = np.asarray(bk, np.float32)
    bv = np.asarray(bv, np.float32)
    in_maps = []
    for c in range(NCORES):
        cs = slice(c * 128, (c + 1) * 128)
        wv_aug = np.zeros((D, 130), np.float32)
        wv_aug[:, 0:64] = wv[:, c * 128:c * 128 + 64]
        wv_aug[:, 65:129] = wv[:, c * 128 + 64:c * 128 + 128]
        bv_aug = np.zeros(130, np.float32)
        bv_aug[0:64] = bv[c * 128:c * 128 + 64]
        bv_aug[64] = 1.0
        bv_aug[65:129] = bv[c * 128 + 64:c * 128 + 128]
        bv_aug[129] = 1.0
        in_maps.append({
            "xT": xT,
            "wq": wq[:, cs].astype(NPBF16),
            "wk": wk[:, cs].astype(NPBF16),
            "wv": wv_aug.astype(NPBF16),
            "bq": np.ascontiguousarray(bq[cs].reshape(128, 1)),
            "bk": np.ascontiguousarray(bk[cs].reshape(128, 1)),
            "bv": np.ascontiguousarray(np.broadcast_to(bv_aug, (128, 130))),
            "wo": wo[cs, :].astype(NPBF16),
        })
    return in_maps


def kernel(x, wq, bq, wk, bk, wv, bv, wo, bo, _run_kwargs=None):
    if "nc" not in _CACHE:
        _CACHE["nc"] = _build_nc()
    nc = _CACHE["nc"]
    in_maps = _prep_in_maps(x, wq, bq, wk, bk, wv, bv, wo)
    res = run_bass_kernel_spmd(nc, in_maps, list(range(NCORES)),
                               **(_run_kwargs or {}))
    acc = np.zeros((NT, D), np.float32)
    for c in range(NCORES):
        acc += res.results[c]["out"]
    acc += np.asarray(bo, np.float32)[None, :]
    if _run_kwargs:
        _CACHE["last_results"] = res
    return acc.reshape(B, S, D)


# revision 5
# speedup vs baseline: 1.0705x; 1.0705x over previous
"""Multi-head self-attention on 8 Trainium2 NeuronCores.

Tensor-parallel over heads: core c owns heads 2c, 2c+1 (128 of the 1024
hidden columns).  Each core:
  1. Qt/Kt = (x @ wq/wk + b)^T  in [d, token] layout (d on partitions,
     2 heads stacked: partitions 0:64 head0, 64:128 head1)
  2. V_aug = x @ [wv | 0] + [bv | 1]  in [token, 65-per-head] layout
     (ones column -> softmax denominator comes out of the P@V matmul)
  3. per (batch, head): S^T tiles = K^T.T @ Q^T  (contraction d=64),
     P^T = exp(S^T / 8) (no max subtraction needed: |S/8| < ~3),
     out^T[65, s] = V_aug.T @ P^T accumulated over t tiles,
     attnT = out^T[0:64] * broadcast(1 / out^T[64])
  4. partial = attnT.T @ wo[128 rows of this core]  -> HBM (f32)
Host sums the 8 partials and adds bo.

Shapes hardcoded for x:[2,2048,1024], 16 heads, d_k=64.
"""

import numpy as np
import ml_dtypes

import concourse.bass as bass
import concourse.tile as tile
from concourse import bacc, mybir
from concourse.bass import ts
from concourse.bass_utils import run_bass_kernel_spmd

BF16 = mybir.dt.bfloat16
F32 = mybir.dt.float32
NPBF16 = ml_dtypes.bfloat16

B = 2
S = 2048
D = 1024
NT = B * S  # 4096 tokens
DK = 64
NCORES = 8
HPC = 2  # heads per core
SC = 1024  # attention s-chunk (exp op free size)

_CACHE = {}


def _build_nc():
    nc = bacc.Bacc("TRN2", target_bir_lowering=False, debug=False,
                   num_devices=NCORES)

    xT = nc.dram_tensor("xT", [D, NT], BF16, kind="ExternalInput").ap()
    wq = nc.dram_tensor("wq", [D, 128], BF16, kind="ExternalInput").ap()
    wk = nc.dram_tensor("wk", [D, 128], BF16, kind="ExternalInput").ap()
    wv = nc.dram_tensor("wv", [D, 130], BF16, kind="ExternalInput").ap()
    bq = nc.dram_tensor("bq", [128, 1], F32, kind="ExternalInput").ap()
    bk = nc.dram_tensor("bk", [128, 1], F32, kind="ExternalInput").ap()
    bv = nc.dram_tensor("bv", [128, 130], F32, kind="ExternalInput").ap()
    wo = nc.dram_tensor("wo", [128, D], BF16, kind="ExternalInput").ap()
    out = nc.dram_tensor("out", [NT, D], F32, kind="ExternalOutput").ap()

    with tile.TileContext(nc) as tc:
        _emit(nc, tc, xT, wq, wk, wv, bq, bk, bv, wo, out)
    nc.compile()
    return nc


def _emit(nc, tc, xT, wq, wk, wv, bq, bk, bv, wo, out):
    import contextlib
    ctx = contextlib.ExitStack()
    with ctx:
        consts = ctx.enter_context(tc.tile_pool(name="consts", bufs=1))
        ptp = ctx.enter_context(tc.tile_pool(name="ptp", bufs=34))
        psp = ctx.enter_context(tc.tile_pool(name="psp", bufs=3, space="PSUM"))
        psb = ctx.enter_context(tc.tile_pool(name="psb", bufs=2, space="PSUM"))
        stg = ctx.enter_context(tc.tile_pool(name="stg", bufs=4))
        nrm = ctx.enter_context(tc.tile_pool(name="nrm", bufs=3))

        # ---- persistent SBUF tensors ----
        xT_sb = consts.tile([128, 8, NT], BF16)      # 8 k-tiles of x^T
        wq_sb = consts.tile([128, 8, 128], BF16)
        wk_sb = consts.tile([128, 8, 128], BF16)
        wv_sb = consts.tile([128, 8, 130], BF16)
        bq_sb = consts.tile([128, 1], F32)
        bk_sb = consts.tile([128, 1], F32)
        bv_sb = consts.tile([128, 130], F32)
        wo_sb = consts.tile([128, D], BF16)
        QT = consts.tile([128, NT], BF16)
        KT = consts.tile([128, NT], BF16)
        V_sb = consts.tile([128, 32, 130], BF16)     # [t-in-tile, t_tile, col]
        attnT = consts.tile([128, NT], BF16)

        xT_d = xT.rearrange("(k p) n -> k p n", p=128)
        wq_d = wq.rearrange("(k p) c -> k p c", p=128)
        wk_d = wk.rearrange("(k p) c -> k p c", p=128)
        wv_d = wv.rearrange("(k p) c -> k p c", p=128)
        for k in range(8):
            nc.sync.dma_start(out=xT_sb[:, k, :], in_=xT_d[k])
            nc.sync.dma_start(out=wq_sb[:, k, :], in_=wq_d[k])
            nc.sync.dma_start(out=wk_sb[:, k, :], in_=wk_d[k])
            nc.sync.dma_start(out=wv_sb[:, k, :], in_=wv_d[k])
        nc.sync.dma_start(out=bq_sb, in_=bq)
        nc.sync.dma_start(out=bk_sb, in_=bk)
        nc.sync.dma_start(out=bv_sb, in_=bv)
        nc.sync.dma_start(out=wo_sb, in_=wo)

        # ---- emit helpers for PE work that can fill ACT-bound phases ----
        def emit_v_tile(tt):
            # V_aug [t,130] += xT[k, t-tile].T @ wv[k]
            psv = psb.tile([128, 512], F32, tag="psb")
            for k in range(8):
                nc.tensor.matmul(psv[:, 0:130], lhsT=xT_sb[:, k, ts(tt, 128)],
                                 rhs=wv_sb[:, k, :],
                                 start=(k == 0), stop=(k == 7))
            nc.vector.tensor_add(V_sb[:, tt, :], psv[:, 0:130], bv_sb)

        def emit_wo_tile(tt):
            for eh in range(2):
                pw = psb.tile([128, 512], F32, tag="psb")
                nc.tensor.matmul(pw, lhsT=attnT[:, ts(tt, 128)],
                                 rhs=wo_sb[:, ts(eh, 512)],
                                 start=True, stop=True)
                ob = stg.tile([128, 512], F32, tag="ob")
                nc.vector.tensor_copy(ob, pw)
                nc.sync.dma_start(
                    out=out[tt * 128:(tt + 1) * 128, eh * 512:(eh + 1) * 512],
                    in_=ob)

        # ---- Q^T / K^T projections: [c,128] += w[k].T @ xT[k] ----
        for w_sb, b_sb, o_sb in ((wq_sb, bq_sb, QT), (wk_sb, bk_sb, KT)):
            for n in range(8):  # 512-token chunks
                ps = psb.tile([128, 512], F32, tag="psb")
                for k in range(8):
                    nc.tensor.matmul(ps, lhsT=w_sb[:, k, :],
                                     rhs=xT_sb[:, k, ts(n, 512)],
                                     start=(k == 0), stop=(k == 7))
                nc.vector.tensor_scalar_add(o_sb[:, ts(n, 512)], ps, b_sb)

        # ---- V_aug projection for batch 0 ----
        for tt in range(16):
            emit_v_tile(tt)

        # ---- attention + output projection ----
        # "extras": independent PE work interleaved into ACT-bound phases
        for b in range(B):
            for sc in range(S // SC):
                if b == 0:
                    extras = [lambda t=t: emit_v_tile(t)
                              for t in range(16 + sc * 8, 16 + (sc + 1) * 8)]
                else:
                    extras = [lambda t=t: emit_wo_tile(t)
                              for t in range(sc * 8, (sc + 1) * 8)]
                s0 = b * S + sc * SC
                pts = []
                for tt in range(16):
                    row = []
                    for h in range(HPC):
                        ps = psp.tile([128, SC], F32, tag="ps")
                        hsl = slice(h * DK, (h + 1) * DK)
                        for n2 in range(SC // 512):
                            nc.tensor.matmul(
                                ps[:, ts(n2, 512)],
                                lhsT=KT[hsl, b * S + tt * 128:b * S + (tt + 1) * 128],
                                rhs=QT[hsl, s0 + n2 * 512:s0 + (n2 + 1) * 512],
                                start=True, stop=True)
                        pt = ptp.tile([128, SC], BF16, tag="pt")
                        nc.scalar.activation(
                            out=pt, in_=ps,
                            func=mybir.ActivationFunctionType.Exp,
                            scale=0.125)
                        row.append(pt)
                    pts.append(row)
                    if tt % 2 == 1 and extras:
                        extras.pop(0)()
                for h in range(HPC):
                    pso = psp.tile([128, SC], F32, tag="ps")
                    for tt in range(16):
                        for n2 in range(SC // 512):
                            nc.tensor.matmul(
                                pso[0:65, ts(n2, 512)],
                                lhsT=V_sb[:, b * 16 + tt, h * 65:(h + 1) * 65],
                                rhs=pts[tt][h][:, ts(n2, 512)],
                                start=(tt == 0), stop=(tt == 15))
                    rec = nrm.tile([1, SC], F32, tag="rec")
                    nc.vector.reciprocal(out=rec, in_=pso[64:65, :])
                    recb = nrm.tile([64, SC], F32, tag="recb")
                    nc.gpsimd.partition_broadcast(recb, rec)
                    nc.vector.tensor_mul(
                        attnT[h * DK:(h + 1) * DK, s0:s0 + SC],
                        pso[0:64, :], recb)
                for e in extras:
                    e()
        # remaining output projection (batch 1 tokens)
        for tt in range(16, 32):
            emit_wo_tile(tt)


def _prep_in_maps(x, wq, bq, wk, bk, wv, bv, wo):
    x2 = np.asarray(x, np.float32).reshape(NT, D)
    xT = np.ascontiguousarray(x2.T).astype(NPBF16)
    wq = np.asarray(wq, np.float32)
    wk = np.asarray(wk, np.float32)
    wv = np.asarray(wv, np.float32)
    wo = np.asarray(wo, np.float32)
    bq = np.asarray(bq, np.float32)
    bk = np.asarray(bk, np.float32)
    bv = np.asarray(bv, np.float32)
    in_maps = []
    for c in range(NCORES):
        cs = slice(c * 128, (c + 1) * 128)
        wv_aug = np.zeros((D, 130), np.float32)
        wv_aug[:, 0:64] = wv[:, c * 128:c * 128 + 64]
        wv_aug[:, 65:129] = wv[:, c * 128 + 64:c * 128 + 128]
        bv_aug = np.zeros(130, np.float32)
        bv_aug[0:64] = bv[c * 128:c * 128 + 64]
        bv_aug[64] = 1.0
        bv_aug[65:129] = bv[c * 128 + 64:c * 128 + 128]
        bv_aug[129] = 1.0
        in_maps.append({
            "xT": xT,
            "wq": wq[:, cs].astype(NPBF16),
            "wk": wk[:, cs].astype(NPBF16),
            "wv": wv_aug.astype(NPBF16),
            "bq": np.ascontiguousarray(bq[cs].reshape(128, 1)),
            "bk": np.ascontiguousarray(bk[cs].reshape(128, 1)),
            "bv": np.ascontiguousarray(np.broadcast_to(bv_aug, (128, 130))),
            "wo": wo[cs, :].astype(NPBF16),
        })
    return in_maps


def kernel(x, wq, bq, wk, bk, wv, bv, wo, bo, _run_kwargs=None):
    if "nc" not in _CACHE:
        _CACHE["nc"] = _build_nc()
    nc = _CACHE["nc"]
    in_maps = _prep_in_maps(x, wq, bq, wk, bk, wv, bv, wo)
    res = run_bass_kernel_spmd(nc, in_maps, list(range(NCORES)),
                               **(_run_kwargs or {}))
    acc = np.zeros((NT, D), np.float32)
    for c in range(NCORES):
        acc += res.results[c]["out"]
    acc += np.asarray(bo, np.float32)[None, :]
    if _run_kwargs:
        _CACHE["last_results"] = res
    return acc.reshape(B, S, D)
